# revision 16
# baseline (speedup 1.0000x reference)
"""Stereo cost-volume generator (nn_CostGenerator) for 8 Trainium2 cores.

cost[b, c, d, h, w] = left[b, c, h, w] - right[b, c, h, w - d]  (0 where w < d)

Sharding: the 64 (B*C) channels are split 8-per-core (data parallel).

The whole device pipeline runs in bf16 (inputs converted on host, output
upcast on host): the rel-err gate is 2e-2 and bf16 end-to-end measures
~4.5e-3.  bf16 halves the output-DMA bytes (the memory-bound critical
path) and qualifies every DVE tensor_sub operand for the 2x_1p fast
mode (2 elem/cycle/lane), halving vector-engine time too.

Inputs ride the Scalar HWDGE ring (channel-0 first so compute starts
early); the Sync ring is reserved for streaming the output.  NOTE: the
output stream runs at ~26 GB/s x 16 DMA engines and the run is
memory-bound end to end, so exec time is sensitive to how the runtime
happens to place DRAM buffers (a collision with the profiler's trace
buffer costs ~12us on one engine).  This exact program layout measured
71.6-71.7us repeatedly; seemingly-neutral variations (splitting DMAs
differently, reshaping input params) re-roll that placement and
measured 81-89us, so the structure below is load-bearing as a whole.

Per channel the 48 disparity slices are computed as 6 groups of 8
rows (e = 47-d, group k = rows e in [8k, 8k+8)) with ONE DVE tensor_sub
per group using an overlapping (Hankel) access pattern on the zero-
padded right image: in1[i, t] = rpad[40 + i + t], against a broadcast
left.  Each group is stored PACKED at its own width W_k = 216 + 8k (the
widest row of the group), which drops the all-zero wedge from both the
DVE work and the DMA bytes (11328 instead of 12288 elems per
partition).  The 168 garbage cells per channel (group-local row i,
cols [0, 7-i)) are skipped by the host unpack, leaving exact zeros
from the np.zeros canvas.

Each channel's packed [h, 11328] tile is streamed out on the Sync ring
in three group-pair DMAs; the host unpacks groups back into the full
[B, C, D, H, W] f32 result.
"""

import numpy as np

B, C, H, W, D = 2, 32, 128, 256, 48
NCORES = 8
CH = (B * C) // NCORES  # channels per core
PW = W + D - 1  # padded right row: 47 zeros + 256 values
NG = D // 8  # 6 groups of 8 e-rows
WK = [216 + 8 * k for k in range(NG)]  # group widths
GOFF = [0]
for k in range(NG):
    GOFF.append(GOFF[-1] + 8 * WK[k])
PACK = GOFF[-1]  # 11328 elems per partition
# out-DMA split points: groups [0-1], [2-3], [4-5]
OUT_SPLITS = [(GOFF[0], GOFF[2]), (GOFF[2], GOFF[4]), (GOFF[4], GOFF[6])]


def _cap(ap, base_off, part_pitch, dims):
    """Custom AP on ap's tensor at ap.offset+base_off; partition dim [pitch, H],
    free dims = list of (stride, size)."""
    import bass_rust

    return bass_rust.AP(
        tensor=ap.tensor,
        offset=ap.offset + base_off,
        ap=bass_rust.VecI64Pair([[part_pitch, H]] + [list(d) for d in dims]),
    )


def _build_nc():
    import concourse.bacc as bacc
    import concourse.mybir as mybir
    from concourse.tile import TileContext

    bf16 = mybir.dt.bfloat16
    nc = bacc.Bacc()
    inp = nc.declare_dram_parameter("inp", [2, CH, H, W], bf16, isOutput=False)
    out = nc.declare_dram_parameter("out", [CH, H, PACK], bf16, isOutput=True)

    with TileContext(nc) as tc:
        with tc.tile_pool(name="io", bufs=1) as pool:
            lt = pool.tile([H, CH * W], bf16, tag="lt", name="lt")
            rp = pool.tile([H, CH * PW], bf16, tag="rp", name="rp")
            obufs = [
                pool.tile([H, PACK], bf16, tag=f"ob{i}", name=f"ob{i}")
                for i in range(3)
            ]

            # zero the 47-col pad strips of all right channels (one 2D memset)
            nc.vector.memset(_cap(rp, 0, CH * PW, [(PW, CH), (1, D - 1)]), 0.0)

            # channel-0 inputs first so compute can start early; all input
            # loads go on the Scalar HWDGE ring so their fixed costs never
            # bubble the Sync ring that streams the output.
            nc.scalar.dma_start(out=lt[:, :W], in_=inp[0][0])
            nc.scalar.dma_start(
                out=_cap(rp, D - 1, CH * PW, [(1, W)]), in_=inp[1][0]
            )
            # remaining channels
            nc.scalar.dma_start(
                out=_cap(lt, W, CH * W, [(W, CH - 1), (1, W)]),
                in_=inp[0][1:].transpose([1, 0, 2]),
            )
            nc.scalar.dma_start(
                out=_cap(rp, PW + D - 1, CH * PW, [(PW, CH - 1), (1, W)]),
                in_=inp[1][1:].transpose([1, 0, 2]),
            )

            for j in range(CH):
                ob = obufs[j % 3]
                for k in range(NG):
                    wk, w0 = WK[k], 40 - 8 * k
                    # ob[h, G_k + i*wk + t] = left[h, w0+t] - rpad[h, 40+i+t]
                    nc.vector.tensor_sub(
                        out=_cap(ob, GOFF[k], PACK, [(wk, 8), (1, wk)]),
                        in0=_cap(lt, j * W + w0, CH * W, [(0, 8), (1, wk)]),
                        in1=_cap(rp, j * PW + 40, CH * PW, [(1, 8), (1, wk)]),
                    )
                for a, b in OUT_SPLITS:
                    nc.sync.dma_start(out=out[j][:, a:b], in_=ob[:, a:b])
    nc.finalize()
    return nc


def _shard_inputs(left_feature, right_feature):
    import ml_dtypes

    bf = ml_dtypes.bfloat16
    lf = np.asarray(left_feature, dtype=np.float32).reshape(B * C, H, W).astype(bf)
    rf = np.asarray(right_feature, dtype=np.float32).reshape(B * C, H, W).astype(bf)
    in_maps = []
    for i in range(NCORES):
        sl = slice(i * CH, (i + 1) * CH)
        in_maps.append({"inp": np.ascontiguousarray(np.stack([lf[sl], rf[sl]]))})
    return in_maps


def _unpack_core(arr):
    # arr: [CH, H, PACK] packed bf16 -> [CH, D, H, W] dense f32 (d-order);
    # garbage cells (row i < 7, cols [0, 7-i)) are skipped so the zero
    # canvas shows through for the w < d wedge.
    cost = np.zeros((arr.shape[0], D, H, W), np.float32)
    for k in range(NG):
        wk, w0 = WK[k], 40 - 8 * k
        blk = arr[:, :, GOFF[k] : GOFF[k + 1]].reshape(arr.shape[0], H, 8, wk)
        for i in range(8):
            d = D - 1 - (8 * k + i)
            g = max(0, 7 - i)
            cost[:, d, :, w0 + g :] = blk[:, :, i, g:].astype(np.float32)
    return cost


def _gather(results):
    parts = [_unpack_core(np.asarray(r["out"])) for r in results]
    cost = np.concatenate(parts, axis=0).reshape(B, C, D, H, W)
    return np.ascontiguousarray(cost)


def kernel(left_feature, right_feature, max_disp_at_scale):
    assert int(max_disp_at_scale) == D, max_disp_at_scale
    from concourse.bass_utils import run_bass_kernel_spmd

    nc = _build_nc()
    in_maps = _shard_inputs(left_feature, right_feature)
    res = run_bass_kernel_spmd(nc, in_maps, core_ids=list(range(NCORES)))
    return _gather(res.results)


# revision 18
# speedup vs baseline: 1.1610x; 1.1610x over previous
"""Stereo cost-volume generator (nn_CostGenerator) for 8 Trainium2 cores.

cost[b, c, d, h, w] = left[b, c, h, w] - right[b, c, h, w - d]  (0 where w < d)

Sharding: the 64 (B*C) channels are split 8-per-core (data parallel).

The whole device pipeline runs in bf16 (inputs converted on host, output
upcast on host): the rel-err gate is 2e-2 and bf16 end-to-end measures
~4.5e-3.  bf16 halves the output-DMA bytes (the memory-bound critical
path) and qualifies every DVE tensor_sub operand for the 2x_1p fast
mode (2 elem/cycle/lane), halving vector-engine time too.

Inputs ride the Scalar HWDGE ring (channel-0 first so compute starts
early); the Sync ring is reserved for streaming the output.  NOTE: the
output stream runs at ~26 GB/s x 16 DMA engines and the run is
memory-bound end to end, so exec time is sensitive to how the runtime
happens to place DRAM buffers (a collision with the profiler's trace
buffer costs ~12us on one engine).  This exact program layout measured
71.6-71.7us repeatedly; seemingly-neutral variations (splitting DMAs
differently, reshaping input params) re-roll that placement and
measured 81-89us, so the structure below is load-bearing as a whole.

Per channel the 48 disparity slices are computed as 6 groups of 8
rows (e = 47-d, group k = rows e in [8k, 8k+8)) with ONE DVE tensor_sub
per group using an overlapping (Hankel) access pattern on the zero-
padded right image: in1[i, t] = rpad[40 + i + t], against a broadcast
left.  Each group is stored PACKED at its own width W_k = 216 + 8k (the
widest row of the group), which drops the all-zero wedge from both the
DVE work and the DMA bytes (11328 instead of 12288 elems per
partition).  The 168 garbage cells per channel (group-local row i,
cols [0, 7-i)) are skipped by the host unpack, leaving exact zeros
from the np.zeros canvas.

Each channel's packed [h, 11328] tile is streamed out on the Sync ring
in three group-pair DMAs; the host unpacks groups back into the full
[B, C, D, H, W] f32 result.
"""

import numpy as np

B, C, H, W, D = 2, 32, 128, 256, 48
NCORES = 8
CH = (B * C) // NCORES  # channels per core
PW = W + D - 1  # padded right row: 47 zeros + 256 values
NG = D // 8  # 6 groups of 8 e-rows
WK = [216 + 8 * k for k in range(NG)]  # group widths
GOFF = [0]
for k in range(NG):
    GOFF.append(GOFF[-1] + 8 * WK[k])
PACK = GOFF[-1]  # 11328 elems per partition
# out-DMA split points: groups [0-1], [2-3], [4-5]
OUT_SPLITS = [(GOFF[0], GOFF[2]), (GOFF[2], GOFF[4]), (GOFF[4], GOFF[6])]


def _cap(ap, base_off, part_pitch, dims):
    """Custom AP on ap's tensor at ap.offset+base_off; partition dim [pitch, H],
    free dims = list of (stride, size)."""
    import bass_rust

    return bass_rust.AP(
        tensor=ap.tensor,
        offset=ap.offset + base_off,
        ap=bass_rust.VecI64Pair([[part_pitch, H]] + [list(d) for d in dims]),
    )


def _build_nc():
    import concourse.bacc as bacc
    import concourse.mybir as mybir
    from concourse.tile import TileContext

    bf16 = mybir.dt.bfloat16
    nc = bacc.Bacc()
    inp = nc.declare_dram_parameter("inp", [2, CH, H, W], bf16, isOutput=False)
    out = nc.declare_dram_parameter("out", [CH, H, PACK], bf16, isOutput=True)

    with TileContext(nc) as tc:
        with tc.tile_pool(name="io", bufs=1) as pool:
            lt = pool.tile([H, CH * W], bf16, tag="lt", name="lt")
            rp = pool.tile([H, CH * PW], bf16, tag="rp", name="rp")
            obufs = [
                pool.tile([H, PACK], bf16, tag=f"ob{i}", name=f"ob{i}")
                for i in range(4)
            ]

            # zero the 47-col pad strips of all right channels (one 2D memset)
            nc.vector.memset(_cap(rp, 0, CH * PW, [(PW, CH), (1, D - 1)]), 0.0)

            # channel-0 inputs first so compute can start early; all input
            # loads go on the Scalar HWDGE ring so their fixed costs never
            # bubble the Sync ring that streams the output.
            nc.scalar.dma_start(out=lt[:, :W], in_=inp[0][0])
            nc.scalar.dma_start(
                out=_cap(rp, D - 1, CH * PW, [(1, W)]), in_=inp[1][0]
            )
            # remaining channels
            nc.scalar.dma_start(
                out=_cap(lt, W, CH * W, [(W, CH - 1), (1, W)]),
                in_=inp[0][1:].transpose([1, 0, 2]),
            )
            nc.scalar.dma_start(
                out=_cap(rp, PW + D - 1, CH * PW, [(PW, CH - 1), (1, W)]),
                in_=inp[1][1:].transpose([1, 0, 2]),
            )

            for j in range(CH):
                ob = obufs[j % 4]
                for k in range(NG):
                    wk, w0 = WK[k], 40 - 8 * k
                    # ob[h, G_k + i*wk + t] = left[h, w0+t] - rpad[h, 40+i+t]
                    nc.vector.tensor_sub(
                        out=_cap(ob, GOFF[k], PACK, [(wk, 8), (1, wk)]),
                        in0=_cap(lt, j * W + w0, CH * W, [(0, 8), (1, wk)]),
                        in1=_cap(rp, j * PW + 40, CH * PW, [(1, 8), (1, wk)]),
                    )
                for a, b in OUT_SPLITS:
                    nc.sync.dma_start(out=out[j][:, a:b], in_=ob[:, a:b])
    nc.finalize()
    return nc


def _shard_inputs(left_feature, right_feature):
    import ml_dtypes

    bf = ml_dtypes.bfloat16
    lf = np.asarray(left_feature, dtype=np.float32).reshape(B * C, H, W).astype(bf)
    rf = np.asarray(right_feature, dtype=np.float32).reshape(B * C, H, W).astype(bf)
    in_maps = []
    for i in range(NCORES):
        sl = slice(i * CH, (i + 1) * CH)
        in_maps.append({"inp": np.ascontiguousarray(np.stack([lf[sl], rf[sl]]))})
    return in_maps


def _unpack_core(arr):
    # arr: [CH, H, PACK] packed bf16 -> [CH, D, H, W] dense f32 (d-order);
    # garbage cells (row i < 7, cols [0, 7-i)) are skipped so the zero
    # canvas shows through for the w < d wedge.
    cost = np.zeros((arr.shape[0], D, H, W), np.float32)
    for k in range(NG):
        wk, w0 = WK[k], 40 - 8 * k
        blk = arr[:, :, GOFF[k] : GOFF[k + 1]].reshape(arr.shape[0], H, 8, wk)
        for i in range(8):
            d = D - 1 - (8 * k + i)
            g = max(0, 7 - i)
            cost[:, d, :, w0 + g :] = blk[:, :, i, g:].astype(np.float32)
    return cost


def _gather(results):
    parts = [_unpack_core(np.asarray(r["out"])) for r in results]
    cost = np.concatenate(parts, axis=0).reshape(B, C, D, H, W)
    return np.ascontiguousarray(cost)


def kernel(left_feature, right_feature, max_disp_at_scale):
    assert int(max_disp_at_scale) == D, max_disp_at_scale
    from concourse.bass_utils import run_bass_kernel_spmd

    nc = _build_nc()
    in_maps = _shard_inputs(left_feature, right_feature)
    res = run_bass_kernel_spmd(nc, in_maps, core_ids=list(range(NCORES)))
    return _gather(res.results)
